# revision 1
# baseline (speedup 1.0000x reference)
import os

import numpy as np

import concourse.bacc as bacc
import concourse.mybir as mybir
from concourse.tile import TileContext, add_dep_helper
from concourse.bass_utils import run_bass_kernel_spmd

F32 = mybir.dt.float32
I32 = mybir.dt.int32
NC = 8
H = 1024
NF = 256
OUT = 1024
GPC = 512
BIG = 1.0e8

_built = {}


def _build(T):
    nc = bacc.Bacc("TRN2", target_bir_lowering=False, debug=False, num_devices=NC)
    A = mybir.ActivationFunctionType

    xt_d = nc.dram_tensor("xt", (3, 128, T), F32, kind="ExternalInput")
    wih_d = nc.dram_tensor("wih", (3, 128, GPC), F32, kind="ExternalInput")
    whh_d = nc.dram_tensor("whh", (8, 128, GPC), F32, kind="ExternalInput")
    wout_d = nc.dram_tensor("wout", (8, 128, OUT), F32, kind="ExternalInput")
    c0_d = nc.dram_tensor("c0", (128, 1), F32, kind="ExternalInput")
    bout_d = nc.dram_tensor("bout", (128, 8), F32, kind="ExternalInput")
    iota_d = nc.dram_tensor("iota1", (128, 8), F32, kind="ExternalInput")

    idx_d = nc.dram_tensor("tour_idx", (1, T), I32, kind="ExternalOutput")
    logp_d = nc.dram_tensor("tour_logp", (1, T), F32, kind="ExternalOutput")
    hdbg_d = nc.dram_tensor("h_dbg", (128, 8), F32, kind="ExternalOutput")

    ag_in = nc.dram_tensor("ag_in", (T, 128), F32, kind="Internal")
    ag_out = nc.dram_tensor(
        "ag_out", (T, 128, 8), F32, kind="Internal", addr_space="Shared"
    )
    rg = [list(range(NC))]

    tch = [(n * 512, min(512, T - n * 512)) for n in range((T + 511) // 512)]

    with TileContext(nc) as tc:
        with (
            tc.tile_pool(name="wts", bufs=1) as wpool,
            tc.tile_pool(name="acts", bufs=1) as apool,
            tc.tile_pool(name="ps", bufs=2, space="PSUM") as ppool,
            tc.tile_pool(name="ps1", bufs=4, space="PSUM") as ppool1,
        ):
            xt = [wpool.tile([128, T], F32, tag=f"xt{c}", name=f"xt{c}") for c in range(3)]
            wih = [wpool.tile([128, GPC], F32, tag=f"wih{c}", name=f"wih{c}") for c in range(3)]
            whh = [wpool.tile([128, GPC], F32, tag=f"whh{c}", name=f"whh{c}") for c in range(8)]
            wout = [wpool.tile([128, OUT], F32, tag=f"wout{c}", name=f"wout{c}") for c in range(8)]
            c0 = wpool.tile([128, 1], F32, tag="c0")
            bout = wpool.tile([128, 8], F32, tag="bout")
            iota1 = wpool.tile([128, 8], F32, tag="iota")
            ones_row = wpool.tile([1, 128], F32, tag="onesr")
            ones_col = wpool.tile([128, 1], F32, tag="onesc")
            negbig = wpool.tile([128, 8], F32, tag="negbig")

            gx = wpool.tile([128, 4, T], F32, tag="gx")
            hT = wpool.tile([128, 8, T], F32, tag="hT")
            h0 = wpool.tile([128, 8], F32, tag="h0")
            scanL = wpool.tile([128, 8, T], F32, tag="scanL")
            zrow = wpool.tile([1, T], F32, tag="zrow")
            esum = wpool.tile([128, T], F32, tag="esum")
            escr = wpool.tile([128, T], F32, tag="escr")

            act = apool.tile([128, 4], F32, tag="act")
            t2 = apool.tile([128, 1], F32, tag="t2")
            cct = apool.tile([128, 1], F32, tag="cct")
            th = apool.tile([128, 1], F32, tag="th")
            hnew = apool.tile([128, 1], F32, tag="hnew")

            v = wpool.tile([128, 8], F32, tag="vis")
            rmax = wpool.tile([128, 32], F32, tag="rmax")
            ftr = wpool.tile([32, 128], F32, tag="ftr")
            mx8 = wpool.tile([1, 8], F32, tag="mx8")
            eq = wpool.tile([128, 8], F32, tag="eq")
            eqm = wpool.tile([128, 8], F32, tag="eqm")
            csum = wpool.tile([128, 1], F32, tag="csum")
            outIdxF = wpool.tile([1, T], F32, tag="outIdxF")
            outIdxI = wpool.tile([1, T], I32, tag="outIdxI")
            outLogp = wpool.tile([1, T], F32, tag="outLogp")

            for c in range(3):
                nc.sync.dma_start(xt[c][:], xt_d[c])
                nc.sync.dma_start(wih[c][:], wih_d[c])
            for c in range(8):
                nc.sync.dma_start(whh[c][:], whh_d[c])
                nc.sync.dma_start(wout[c][:], wout_d[c])
            nc.sync.dma_start(c0[:], c0_d[:])
            nc.sync.dma_start(bout[:], bout_d[:])
            nc.sync.dma_start(iota1[:], iota_d[:])
            nc.vector.memset(ones_row[:], 1.0)
            nc.vector.memset(ones_col[:], 1.0)
            nc.vector.memset(negbig[:], -BIG)
            nc.vector.memset(h0[:], 0.0)
            nc.vector.memset(rmax[:], 0.0)
            nc.vector.memset(v[:], 0.0)

            PH = int(os.environ.get("KPHASE", "5"))
            P4L = int(os.environ.get("KP4", "7"))
            for m in range(4):
                for n0, nn in tch:
                    pg = ppool.tile([128, 512], F32, tag="pbig")
                    for c in range(3):
                        nc.tensor.matmul(
                            pg[:, :nn],
                            wih[c][:, m * 128:(m + 1) * 128],
                            xt[c][:, n0:n0 + nn],
                            start=(c == 0),
                            stop=(c == 2),
                        )
                    nc.vector.tensor_copy(gx[:, m, n0:n0 + nn], pg[:, :nn])

            nc.vector.memset(hT[:], 0.0)
            nc.vector.memset(scanL[:], 0.01)
            nc.vector.memset(zrow[:], 1.0)
            nc.vector.memset(outIdxF[:], 0.0)
            nc.vector.memset(outLogp[:], 0.0)
            for s in range(T if PH >= 2 else 0):
                pg = ppool.tile([128, 4], F32, tag="p2")
                for m in range(4):
                    for c in range(8):
                        rhs = h0[:, c:c + 1] if s == 0 else hT[:, c, s - 1:s]
                        nc.tensor.matmul(
                            pg[:, m:m + 1],
                            whh[c][:, m * 128:(m + 1) * 128],
                            rhs,
                            start=(c == 0),
                            stop=(c == 7),
                        )
                nc.scalar.activation(
                    act[:, 0:1], pg[:, 0:1], A.Sigmoid, bias=gx[:, 0, s:s + 1])
                nc.scalar.activation(
                    act[:, 1:2], pg[:, 1:2], A.Sigmoid, bias=gx[:, 1, s:s + 1])
                nc.scalar.activation(
                    act[:, 2:3], pg[:, 2:3], A.Sigmoid, bias=gx[:, 2, s:s + 1])
                nc.scalar.activation(
                    act[:, 3:4], pg[:, 3:4], A.Tanh, bias=gx[:, 3, s:s + 1])
                nc.vector.tensor_mul(t2[:], act[:, 0:1], act[:, 3:4])
                nc.vector.scalar_tensor_tensor(
                    cct[:], act[:, 1:2], c0[:], t2[:],
                    op0=mybir.AluOpType.mult, op1=mybir.AluOpType.add)
                nc.scalar.activation(th[:], cct[:], A.Tanh)
                nc.vector.tensor_mul(hnew[:], act[:, 2:3], th[:])
                d_out = nc.sync.dma_start(ag_in[s:s + 1, :], hnew[:])
                cci = nc.gpsimd.collective_compute(
                    "AllGather",
                    mybir.AluOpType.bypass,
                    ins=[ag_in[s:s + 1, :]],
                    outs=[ag_out[s:s + 1]],
                    replica_groups=rg,
                )
                add_dep_helper(cci.ins, d_out.ins, sync=True,
                               reason="ag after dma out")
                d_in = nc.sync.dma_start(hT[:, :, s], ag_out[s])
                add_dep_helper(d_in.ins, cci.ins, sync=True,
                               reason="dma in after ag")

            nc.sync.dma_start(hdbg_d[:], hT[:, :, T - 1])

            for g in range(8 if PH >= 3 else 0):
                for n0, nn in tch:
                    pl = ppool.tile([128, 512], F32, tag="pbig")
                    for c in range(8):
                        nc.tensor.matmul(
                            pl[:, :nn],
                            wout[c][:, g * 128:(g + 1) * 128],
                            hT[:, c, n0:n0 + nn],
                            start=(c == 0),
                            stop=(c == 7),
                        )
                    nc.vector.tensor_scalar_add(
                        scanL[:, g, n0:n0 + nn], pl[:, :nn], bout[:, g:g + 1])

            for g in range(8 if PH >= 4 else 0):
                nc.scalar.activation(escr[:], scanL[:, g, :], A.Exp)
                if g == 0:
                    nc.vector.tensor_copy(esum[:], escr[:])
                else:
                    nc.vector.tensor_add(esum[:], esum[:], escr[:])
            for n0, nn in (tch if PH >= 4 else []):
                pz = ppool1.tile([1, 512], F32, tag="psmall")
                nc.tensor.matmul(
                    pz[:, :nn], ones_col[:], esum[:, n0:n0 + nn],
                    start=True, stop=True)
                nc.scalar.activation(zrow[:, n0:n0 + nn], pz[:, :nn], A.Ln)

            for s in range(T if PH >= 5 else 0):
              if True:
                nc.vector.tensor_add(eqm[:], scanL[:, :, s], v[:])
                nc.vector.reduce_max(
                    rmax[:, 0:1], eqm[:], axis=mybir.AxisListType.X)
                if P4L < 2:
                    continue
                nc.vector.transpose(ftr[:, 0:32], rmax[0:32, 0:32])
                nc.vector.transpose(ftr[:, 32:64], rmax[32:64, 0:32])
                nc.vector.transpose(ftr[:, 64:96], rmax[64:96, 0:32])
                nc.vector.transpose(ftr[:, 96:128], rmax[96:128, 0:32])
                if P4L < 3:
                    continue
                nc.vector.max(out=mx8[:], in_=ftr[0:1, 0:128])
                if P4L < 4:
                    continue
                pmb = ppool1.tile([128, 1], F32, tag="psmall")
                nc.tensor.matmul(pmb[:], ones_row[:], mx8[0:1, 0:1],
                                 start=True, stop=True)
                if P4L < 5:
                    continue
                nc.vector.scalar_tensor_tensor(
                    eq[:], eqm[:], pmb[:], iota1[:],
                    op0=mybir.AluOpType.is_ge, op1=mybir.AluOpType.mult,
                    accum_out=csum[:])
                if P4L < 6:
                    continue
                pcity = ppool1.tile([1, 1], F32, tag="psmall")
                nc.tensor.matmul(pcity[:], csum[:], ones_col[:, 0:1],
                                 start=True, stop=True)
                nc.vector.tensor_scalar_add(outIdxF[:, s:s + 1], pcity[:], -1.0)
                nc.vector.tensor_sub(
                    outLogp[:, s:s + 1], mx8[0:1, 0:1], zrow[:, s:s + 1])
                if P4L < 7:
                    continue
                nc.vector.scalar_tensor_tensor(
                    eqm[:], eq[:], 0.5, negbig[:],
                    op0=mybir.AluOpType.is_ge, op1=mybir.AluOpType.mult)
                nc.vector.tensor_add(v[:], v[:], eqm[:])

            nc.vector.tensor_copy(outIdxI[:], outIdxF[:])
            nc.sync.dma_start(idx_d[:], outIdxI[:])
            nc.sync.dma_start(logp_d[:], outLogp[:])

    nc.compile()
    return nc


def _prep_inputs(input, encoder_output, W_ih, W_hh, b_ih, b_hh, W_out, b_out):
    X = np.asarray(input, dtype=np.float32)
    T = X.shape[0]
    enc = np.asarray(encoder_output, dtype=np.float32)
    W_ih = np.asarray(W_ih, dtype=np.float32)
    W_hh = np.asarray(W_hh, dtype=np.float32)
    b = (np.asarray(b_ih, dtype=np.float32)
         + np.asarray(b_hh, dtype=np.float32))
    W_out = np.asarray(W_out, dtype=np.float32)
    b_out = np.asarray(b_out, dtype=np.float32)

    c0 = enc[-1]

    Xa = np.concatenate(
        [X, np.ones((T, 1), np.float32), np.zeros((T, 127), np.float32)],
        axis=1)
    xt = np.ascontiguousarray(Xa.T.reshape(3, 128, T))

    wout_arr = np.ascontiguousarray(
        W_out.T.reshape(128, 8, OUT).transpose(1, 0, 2))
    bout_arr = np.ascontiguousarray(b_out.reshape(8, 128).T)
    iota_arr = np.ascontiguousarray(
        (np.arange(8)[None, :] * 128 + np.arange(128)[:, None] + 1)
        .astype(np.float32))

    in_maps = []
    for r in range(NC):
        sl = np.arange(r * 128, (r + 1) * 128)
        rows = np.concatenate([sl, H + sl, 3 * H + sl, 2 * H + sl])
        Wia = np.concatenate(
            [W_ih[rows], b[rows][:, None],
             np.zeros((GPC, 127), np.float32)], axis=1)
        wih_arr = np.ascontiguousarray(Wia.T.reshape(3, 128, GPC))
        whh_arr = np.ascontiguousarray(
            W_hh[rows].T.reshape(128, 8, GPC).transpose(1, 0, 2))
        c0_arr = np.ascontiguousarray(c0[sl][:, None])
        in_maps.append({
            "xt": xt,
            "wih": wih_arr,
            "whh": whh_arr,
            "wout": wout_arr,
            "c0": c0_arr,
            "bout": bout_arr,
            "iota1": iota_arr,
        })
    return in_maps


def kernel(input, encoder_output, W_ih, W_hh, b_ih, b_hh, W_out, b_out):
    T = np.asarray(input).shape[0]
    if T not in _built:
        _built[T] = _build(T)
    nc = _built[T]
    in_maps = _prep_inputs(
        input, encoder_output, W_ih, W_hh, b_ih, b_hh, W_out, b_out)
    res = run_bass_kernel_spmd(nc, in_maps, core_ids=list(range(NC)))
    out = res.results[0]
    return (np.asarray(out["tour_idx"]).astype(np.int32),
            np.asarray(out["tour_logp"]).astype(np.float32))

